# revision 24
# baseline (speedup 1.0000x reference)
"""Trainium2 Bass kernel for nn_LoRALinear4bit.

Computes  out = x @ dequant_nf4(q_idx, absmax).T + (x @ A) @ B * 2.0
with x [4,2048,4096] f32, q_idx [4096,4096] int32 (NF4 codes),
absmax [4096,64] f32 (per-64-block scales), A [4096,16], B [16,4096].

Strategy (column / tensor parallel over 8 NeuronCores):
  * shard out_features OUT=4096 into 8 x 512; replicate x, A.
  * all matmul operands in fp16: x (host-cast), dequantized W, LoRA
    factors.  fp16 matmuls run at 1 cyc/row with FastWeightLoad (the
    fp32r path measured 389ns/mm + 187ns unhidden LDWEIGHTS); psum
    accumulation stays fp32.  End-to-end rel err ~3e-3 (gate 2e-2).
  * per core, on device:
      - dequantize W^T shard [IN, 512] from host-transposed int8 codes
        via a 4-exponential + linear fit of the NF4 codebook (rms
        1.6e-3): four ACT Exp passes (one table, loaded once) plus a
        handful of 2x-packed fp16 DVE tensor_tensor ops, times the
        absmax expansion, in [128, 1024] supertiles.
      - fold the LoRA product in: W_eff = poly*scale + 2*(A @ B).
      - out_shard[8192, 512] = x @ W_eff over 32 K-tiles of 128.
  * host casts the gathered fp16 shards back to f32.

Schedule: the PE consumes weff K-tiles as the dequant pipeline emits
them.  Early token-groups accumulate independent k-range chunks in PSUM,
spill to SBUF f32, and combine on the DVE (no PE reinjection); chunk
emission is ordered by the last weff tile needed so the PE never
head-of-line blocks on a tile still being dequantized (which would
also drop the PE into its half-clock HAM state).  Each supertile's
ACT/PE/DVE/gpsimd dequant work is emitted just ahead of the first
matmul that consumes it, keeping every engine stream in prompt
dependency order (Tile tracks deps by emission order).
"""

import numpy as np

# problem shape (hardcoded per contract: kernel.py must be self-contained)
B_, S_, IN, OUT = 4, 2048, 4096, 4096
TOK = B_ * S_            # 8192 tokens
NCORES = 8
OSH = OUT // NCORES      # 512 out-features per core
R = 16                   # LoRA rank
SCALING = 2.0            # alpha/r = 32/16
QBLOCK = 64              # bnb absmax blocksize

KT = IN // 128           # 32 K tiles
TG = 512                 # token group per x DMA
NG = TOK // TG           # 16 token groups
MPG = TG // 128          # 4 m-tiles per group

# dequant supertiles: (first K-tile, width); narrow ones first for a
# quicker first-weff latency, then pairs to amortize per-op overhead
SUPERS = ([(k, 1) for k in range(4)]
          + [(4 + 2 * i, 2) for i in range(14)])
AHEAD = 4                # weff emission lookahead (in K-tiles)

# chunk boundaries (in K-tiles) for the supply-limited early groups;
# groups not listed accumulate all 32 K-tiles in one PSUM pass.
# Group 15 is emitted last: a short final chunk shrinks the drain tail.
CHUNKS = {
    0: [0, 2, 4, 6, 8, 12, 16, 24, 32],
    1: [0, 4, 8, 12, 16, 24, 32],
    2: [0, 8, 16, 24, 32],
    3: [0, 16, 32],
    15: [0, 28, 32],
}

# 4-exponential + linear fit of the NF4 codebook in u = (q-7.5)/7.5:
# NF4[q] ~= sum_i a_i*exp(b_i*u) + c*u + d   (rms 1.6e-3).
# Terms: a near-sinh pair plus one spike per tail.  Amplitudes fold into
# the exp biases (exp(b*u + ln|a|)), so every ACT Exp gets arg <= ~0 and
# fp16 outputs stay in range.
P4EXP = [0.22356227478165563, 1.5166052916459616,
         -1.4431758163675317, -0.6552135545514064,
         1.266508374101154e-14, 29.999999955034003,
         -1.7659087042752649e-06, -11.58303331708526,
         -0.6619131266814475, 1.2573155188954936]

# bitsandbytes NF4 codebook
NF4 = np.array([
    -1.0, -0.6961928009986877, -0.5250730514526367, -0.39491748809814453,
    -0.28444138169288635, -0.18477343022823334, -0.09105003625154495, 0.0,
    0.07958029955625534, 0.16093020141124725, 0.24611230194568634,
    0.33791524171829224, 0.44070982933044434, 0.5626170039176941,
    0.6989699602127075, 1.0], dtype=np.float64)


_CACHE = {}


def _build():
    """Build + compile the per-core Bass program (identical on all cores)."""
    key = "v6"
    if key in _CACHE:
        return _CACHE[key]

    import math
    import concourse.bacc as bacc
    import concourse.tile as tile
    from concourse import mybir
    from concourse.bass import ts, ds

    f16 = mybir.dt.float16
    f32 = mybir.dt.float32
    i8 = mybir.dt.int8
    Alu = mybir.AluOpType
    Act = mybir.ActivationFunctionType

    # exp term i: exp(q*escale_i + ebias_i) with sign esign_i;
    # linear term: q*lscale + lbias  (u = q/7.5 - 1 folded in)
    esign, escale, ebias = [], [], []
    for i in range(4):
        a, b = P4EXP[2 * i], P4EXP[2 * i + 1]
        esign.append(1.0 if a > 0 else -1.0)
        escale.append(b / 7.5)
        ebias.append(-b + math.log(abs(a)))
    lscale = P4EXP[8] / 7.5
    lbias = P4EXP[9] - P4EXP[8]

    NSW = len(SUPERS)        # number of supertiles

    nc = bacc.Bacc("TRN2", target_bir_lowering=False, debug=False)

    xt = nc.dram_tensor("xt", [IN, TOK], f16, kind="ExternalInput").ap()
    qt = nc.dram_tensor("qt", [IN, OSH], i8, kind="ExternalInput").ap()
    scl = nc.dram_tensor("scl", [IN, OSH], f16, kind="ExternalInput").ap()
    at = nc.dram_tensor("at", [R, IN], f16, kind="ExternalInput").ap()
    bsh = nc.dram_tensor("bsh", [R, OSH], f16, kind="ExternalInput").ap()
    out = nc.dram_tensor("out", [TOK, OSH], f16, kind="ExternalOutput").ap()

    with tile.TileContext(nc) as tc:
        with (
            tc.tile_pool(name="weff", bufs=1) as weff_pool,
            tc.tile_pool(name="wadd2", bufs=2) as wadd2_pool,
            tc.tile_pool(name="deq", bufs=3) as deq_pool,
            tc.tile_pool(name="part", bufs=1) as part_pool,
            tc.tile_pool(name="xin", bufs=8) as x_pool,
            tc.tile_pool(name="oup", bufs=4) as o_pool,
            tc.tile_pool(name="wadd_ps", bufs=2, space="PSUM") as wadd_pool,
            tc.tile_pool(name="mm_ps", bufs=6, space="PSUM") as mm_pool,
            tc.tile_pool(name="const", bufs=1) as const_pool,
        ):
            # resident constants
            b_sb = const_pool.tile([R, OSH], f16, tag="b_sb", name="b_sb")
            nc.gpsimd.dma_start(out=b_sb[:], in_=bsh[:])
            at_sb = const_pool.tile([R, IN], f16, tag="at_sb", name="at_sb")
            nc.gpsimd.dma_start(out=at_sb[:], in_=at[:])
            ebias_t = []
            for i in range(4):
                bt = const_pool.tile([128, 1], f32, tag=f"ebias{i}",
                                     name=f"ebias{i}")
                nc.gpsimd.memset(bt[:], ebias[i])
                ebias_t.append(bt)

            weff_s = []
            weff = [None] * KT
            for j, (k0s, sw) in enumerate(SUPERS):
                w = weff_pool.tile([128, sw * OSH], f16, tag=f"weff{j}",
                                   name=f"weff{j}")
                weff_s.append(w)
                for s in range(sw):
                    weff[k0s + s] = w[:, ts(s, OSH)]

            # q/scl DMAs up front (deep-buffered; WAR edges pace reuse)
            qtls, sctls = [], []
            for j, (k0s, sw) in enumerate(SUPERS):
                qtl = deq_pool.tile([128, sw * OSH], i8, tag=f"qtile{sw}",
                                    name="qtl")
                sctl = deq_pool.tile([128, sw * OSH], f16, tag=f"sctile{sw}",
                                     name="sctl")
                for s in range(sw):
                    k = k0s + s
                    nc.gpsimd.dma_start(out=qtl[:, ts(s, OSH)],
                                        in_=qt[ts(k, 128), :])
                    nc.gpsimd.dma_start(out=sctl[:, ts(s, OSH)],
                                        in_=scl[ts(k, 128), :])
                qtls.append(qtl)
                sctls.append(sctl)

            def emit_supertile(j):
                """Emit the full dequant chain for supertile j:
                4x Exp (ACT), linear (DVE TS), LoRA wadd (PE) + x2 copies
                (ACT), sum + scale (DVE TT), weff fold (gpsimd)."""
                k0s, sw = SUPERS[j]
                Wj = sw * OSH
                es = []
                for i in range(4):
                    e = deq_pool.tile([128, Wj], f16, tag=f"e{i}_{sw}",
                                      name=f"e{i}")
                    nc.scalar.activation(e[:], qtls[j][:], Act.Exp,
                                         bias=ebias_t[i][:],
                                         scale=escale[i])
                    es.append(e)
                # LoRA: wadd2 = 2*(A@B) k-slab, via PE psum + ACT copy
                wadd2 = wadd2_pool.tile([128, Wj], f16, tag=f"wadd2_{sw}",
                                        name="wadd2")
                for s in range(sw):
                    k = k0s + s
                    p = wadd_pool.tile([128, OSH], f32, tag="waddps",
                                       name="waddps")
                    nc.tensor.matmul(p[:], at_sb[:, ts(k, 128)], b_sb[:],
                                     start=True, stop=True)
                    nc.scalar.activation(wadd2[:, ts(s, OSH)], p[:],
                                         Act.Copy, bias=0.0, scale=SCALING)
                # linear term on DVE (int8 in, fp16 out)
                lin = deq_pool.tile([128, Wj], f16, tag=f"lin{sw}",
                                    name="lin")
                nc.vector.tensor_scalar(lin[:], qtls[j][:], lscale, lbias,
                                        Alu.mult, Alu.add)
                # combine: acc = (pos exps) - (neg exps) + lin, on DVE
                pos = [es[i] for i in range(4) if esign[i] > 0]
                neg = [es[i] for i in range(4) if esign[i] < 0]
                nc.vector.tensor_tensor(pos[0][:], pos[0][:], pos[1][:],
                                        Alu.add)
                nc.vector.tensor_tensor(neg[0][:], neg[0][:], neg[1][:],
                                        Alu.add)
                nc.vector.tensor_tensor(pos[0][:], pos[0][:], neg[0][:],
                                        Alu.subtract)
                nc.vector.tensor_tensor(pos[0][:], pos[0][:], lin[:],
                                        Alu.add)
                # acc2 = acc * absmax_expanded
                acc2 = deq_pool.tile([128, Wj], f16, tag=f"acc2_{sw}",
                                     name="acc2")
                nc.vector.tensor_tensor(acc2[:], pos[0][:], sctls[j][:],
                                        Alu.mult)
                # weff = acc2 + wadd2  (gpsimd; frees DVE for next chain)
                nc.gpsimd.tensor_add(weff_s[j][:], acc2[:], wadd2[:])

            pending = list(range(NSW))

            def ensure_weff(k_need):
                while pending and SUPERS[pending[0]][0] <= k_need + AHEAD:
                    emit_supertile(pending.pop(0))

            # ---- Phase B: out[g*512+m*128 : ..., :] = x @ W_eff
            partials = {}

            def emit_chunk(g, ci, k0, k1, last):
                """Each chunk accumulates its own k-range in PSUM
                (start=True at k0); partial results combine on the DVE in
                f32 — no PE reinjection matmuls."""
                psums = [mm_pool.tile([128, OSH], f32, tag="mmps",
                                      name="mmps") for _ in range(MPG)]
                for k in range(k0, k1):
                    ensure_weff(k)
                    xg = x_pool.tile([128, TG], f16, tag="xg", name="xg")
                    nc.sync.dma_start(out=xg[:], in_=xt[ts(k, 128), ts(g, TG)])
                    for m in range(MPG):
                        nc.tensor.matmul(
                            psums[m][:],
                            xg[:, ts(m, 128)],
                            weff[k][:],
                            start=(k == k0),
                            stop=(k == k1 - 1))
                for m in range(MPG):
                    if ci == 0 and not last:
                        # first spill: copies split ACT/DVE by parity so
                        # neither supply engine eats the whole burst
                        pt = part_pool.tile([128, OSH], f32,
                                            tag=f"part{g}_{m}",
                                            name=f"part{g}_{m}")
                        partials[(g, m)] = pt
                        if m % 2 == 0:
                            nc.scalar.copy(pt[:], psums[m][:])
                        else:
                            nc.vector.tensor_scalar(pt[:], psums[m][:],
                                                    1.0, 0.0,
                                                    Alu.mult, Alu.add)
                    elif not last:
                        # accumulate into the partial (DVE, f32)
                        nc.vector.tensor_tensor(
                            partials[(g, m)][:], partials[(g, m)][:],
                            psums[m][:], Alu.add)
                    else:
                        ot = o_pool.tile([128, OSH], f16, tag="ot", name="ot")
                        if ci > 0:
                            nc.vector.tensor_tensor(
                                ot[:], psums[m][:], partials[(g, m)][:],
                                Alu.add)
                        elif m % 2 == 0:
                            nc.scalar.copy(ot[:], psums[m][:])
                        else:
                            nc.vector.tensor_scalar(ot[:], psums[m][:],
                                                    1.0, 0.0,
                                                    Alu.mult, Alu.add)
                        nc.scalar.dma_start(
                            out=out[ds(g * TG + m * 128, 128), :], in_=ot[:])

            def emit_ham_warmers(n):
                """Scratch matmuls at known supply-stall points: the PE
                executes them in the gap instead of idling, which keeps
                the HAM activity monitor from halving the PE clock."""
                for _ in range(n):
                    p = wadd_pool.tile([128, OSH], f32, tag="waddps",
                                       name="hamw")
                    nc.tensor.matmul(p[:], at_sb[:, 0:128], b_sb[:],
                                     start=True, stop=True)

            chunks = []      # (k1, g, ci, k0, last)
            for g in range(NG):
                b = CHUNKS.get(g, [0, KT])
                for ci in range(len(b) - 1):
                    chunks.append((b[ci + 1], g, ci, b[ci],
                                   ci == len(b) - 2))
            for k1, g, ci, k0, last in sorted(chunks):
                emit_chunk(g, ci, k0, k1, last)
                if k1 <= 20:
                    emit_ham_warmers(3)
            assert not pending

    nc.compile()
    _CACHE[key] = nc
    return nc


def _prepare_in_maps(x, q_idx, absmax, lora_A, lora_B):
    x = np.asarray(x, dtype=np.float32)
    q_idx = np.asarray(q_idx, dtype=np.int32)
    absmax = np.asarray(absmax, dtype=np.float32)
    lora_A = np.asarray(lora_A, dtype=np.float32)
    lora_B = np.asarray(lora_B, dtype=np.float32)

    xt = np.ascontiguousarray(x.reshape(TOK, IN).T.astype(np.float16))
    qt_full = q_idx.astype(np.int8).T                        # [IN, OUT] view
    at = np.ascontiguousarray(lora_A.T.astype(np.float16))   # [R, IN]

    in_maps = []
    for cid in range(NCORES):
        sl = slice(cid * OSH, (cid + 1) * OSH)
        scale = np.repeat(np.ascontiguousarray(absmax[sl].T), QBLOCK, axis=0)
        in_maps.append({
            "xt": xt,
            "qt": np.ascontiguousarray(qt_full[:, sl]),
            "scl": np.ascontiguousarray(scale.astype(np.float16)),
            "at": at,
            "bsh": np.ascontiguousarray(lora_B[:, sl].astype(np.float16)),
        })
    return in_maps


def _gather(results):
    shards = [results[cid]["out"] for cid in range(NCORES)]
    full = np.concatenate(shards, axis=1)                    # [TOK, OUT]
    return full.astype(np.float32).reshape(B_, S_, OUT)


def kernel(x, q_idx, absmax, lora_A, lora_B):
    from concourse.bass_utils import run_bass_kernel_spmd

    nc = _build()
    in_maps = _prepare_in_maps(x, q_idx, absmax, lora_A, lora_B)
    res = run_bass_kernel_spmd(nc, in_maps, list(range(NCORES)))
    return _gather(res.results)


# revision 27
# speedup vs baseline: 1.0231x; 1.0231x over previous
"""Trainium2 Bass kernel for nn_LoRALinear4bit.

Computes  out = x @ dequant_nf4(q_idx, absmax).T + (x @ A) @ B * 2.0
with x [4,2048,4096] f32, q_idx [4096,4096] int32 (NF4 codes),
absmax [4096,64] f32 (per-64-block scales), A [4096,16], B [16,4096].

Strategy (column / tensor parallel over 8 NeuronCores):
  * shard out_features OUT=4096 into 8 x 512; replicate x, A.
  * all matmul operands in fp16: x (host-cast), dequantized W, LoRA
    factors.  fp16 matmuls run at 1 cyc/row with FastWeightLoad (the
    fp32r path measured 389ns/mm + 187ns unhidden LDWEIGHTS); psum
    accumulation stays fp32.  End-to-end rel err ~3e-3 (gate 2e-2).
  * per core, on device:
      - dequantize W^T shard [IN, 512] from host-transposed int8 codes
        via a 4-exponential + linear fit of the NF4 codebook (rms
        1.6e-3): four ACT Exp passes (one table, loaded once) plus a
        handful of 2x-packed fp16 DVE tensor_tensor ops, times the
        absmax expansion, in [128, 1024] supertiles.
      - fold the LoRA product in: W_eff = poly*scale + 2*(A @ B).
      - out_shard[8192, 512] = x @ W_eff over 32 K-tiles of 128.
  * host casts the gathered fp16 shards back to f32.

Schedule: the PE consumes weff K-tiles as the dequant pipeline emits
them.  Early token-groups accumulate independent k-range chunks in PSUM,
spill to SBUF f32, and combine on the DVE (no PE reinjection); chunk
emission is ordered by the last weff tile needed so the PE never
head-of-line blocks on a tile still being dequantized (which would
also drop the PE into its half-clock HAM state).  Each supertile's
ACT/PE/DVE/gpsimd dequant work is emitted just ahead of the first
matmul that consumes it, keeping every engine stream in prompt
dependency order (Tile tracks deps by emission order).
"""

import numpy as np

# problem shape (hardcoded per contract: kernel.py must be self-contained)
B_, S_, IN, OUT = 4, 2048, 4096, 4096
TOK = B_ * S_            # 8192 tokens
NCORES = 8
OSH = OUT // NCORES      # 512 out-features per core
R = 16                   # LoRA rank
SCALING = 2.0            # alpha/r = 32/16
QBLOCK = 64              # bnb absmax blocksize

KT = IN // 128           # 32 K tiles
TG = 512                 # token group per x DMA
NG = TOK // TG           # 16 token groups
MPG = TG // 128          # 4 m-tiles per group

# dequant supertiles: (first K-tile, width); narrow ones first for a
# quicker first-weff latency, then pairs to amortize per-op overhead
SUPERS = ([(k, 1) for k in range(4)]
          + [(4 + 2 * i, 2) for i in range(14)])
AHEAD = 4                # weff emission lookahead (in K-tiles)

# chunk boundaries (in K-tiles) for the supply-limited early groups;
# groups not listed accumulate all 32 K-tiles in one PSUM pass.
# Group 15 is emitted last: a short final chunk shrinks the drain tail.
CHUNKS = {
    0: [0, 2, 4, 6, 8, 12, 16, 24, 32],
    1: [0, 4, 8, 12, 16, 24, 32],
    2: [0, 8, 16, 24, 32],
    3: [0, 16, 32],
    15: [0, 28, 32],
}

# 4-exponential + linear fit of the NF4 codebook in u = (q-7.5)/7.5:
# NF4[q] ~= sum_i a_i*exp(b_i*u) + c*u + d   (rms 1.6e-3).
# Terms: a near-sinh pair plus one spike per tail.  Amplitudes fold into
# the exp biases (exp(b*u + ln|a|)), so every ACT Exp gets arg <= ~0 and
# fp16 outputs stay in range.
P4EXP = [0.22356227478165563, 1.5166052916459616,
         -1.4431758163675317, -0.6552135545514064,
         1.266508374101154e-14, 29.999999955034003,
         -1.7659087042752649e-06, -11.58303331708526,
         -0.6619131266814475, 1.2573155188954936]

# bitsandbytes NF4 codebook
NF4 = np.array([
    -1.0, -0.6961928009986877, -0.5250730514526367, -0.39491748809814453,
    -0.28444138169288635, -0.18477343022823334, -0.09105003625154495, 0.0,
    0.07958029955625534, 0.16093020141124725, 0.24611230194568634,
    0.33791524171829224, 0.44070982933044434, 0.5626170039176941,
    0.6989699602127075, 1.0], dtype=np.float64)


_CACHE = {}


def _build():
    """Build + compile the per-core Bass program (identical on all cores)."""
    key = "v7"
    if key in _CACHE:
        return _CACHE[key]

    import math
    import concourse.bacc as bacc
    import concourse.tile as tile
    from concourse import mybir
    from concourse.bass import ts, ds

    f16 = mybir.dt.float16
    f32 = mybir.dt.float32
    i8 = mybir.dt.int8
    Alu = mybir.AluOpType
    Act = mybir.ActivationFunctionType

    # exp term i: exp(q*escale_i + ebias_i) with sign esign_i;
    # linear term: q*lscale + lbias  (u = q/7.5 - 1 folded in)
    esign, escale, ebias = [], [], []
    for i in range(4):
        a, b = P4EXP[2 * i], P4EXP[2 * i + 1]
        esign.append(1.0 if a > 0 else -1.0)
        escale.append(b / 7.5)
        ebias.append(-b + math.log(abs(a)))
    lscale = P4EXP[8] / 7.5
    lbias = P4EXP[9] - P4EXP[8]

    NSW = len(SUPERS)        # number of supertiles

    nc = bacc.Bacc("TRN2", target_bir_lowering=False, debug=False)

    xt = nc.dram_tensor("xt", [IN, TOK], f16, kind="ExternalInput").ap()
    qt = nc.dram_tensor("qt", [IN, OSH], i8, kind="ExternalInput").ap()
    scl = nc.dram_tensor("scl", [IN, OSH], f16, kind="ExternalInput").ap()
    at = nc.dram_tensor("at", [R, IN], f16, kind="ExternalInput").ap()
    bsh = nc.dram_tensor("bsh", [R, OSH], f16, kind="ExternalInput").ap()
    out = nc.dram_tensor("out", [TOK, OSH], f16, kind="ExternalOutput").ap()

    with tile.TileContext(nc) as tc:
        with (
            tc.tile_pool(name="weff", bufs=1) as weff_pool,
            tc.tile_pool(name="wadd2", bufs=2) as wadd2_pool,
            tc.tile_pool(name="deq", bufs=3) as deq_pool,
            tc.tile_pool(name="part", bufs=1) as part_pool,
            tc.tile_pool(name="xin", bufs=8) as x_pool,
            tc.tile_pool(name="oup", bufs=4) as o_pool,
            tc.tile_pool(name="wadd_ps", bufs=2, space="PSUM") as wadd_pool,
            tc.tile_pool(name="mm_ps", bufs=6, space="PSUM") as mm_pool,
            tc.tile_pool(name="const", bufs=1) as const_pool,
        ):
            # resident constants
            b_sb = const_pool.tile([R, OSH], f16, tag="b_sb", name="b_sb")
            nc.gpsimd.dma_start(out=b_sb[:], in_=bsh[:])
            at_sb = const_pool.tile([R, IN], f16, tag="at_sb", name="at_sb")
            nc.gpsimd.dma_start(out=at_sb[:], in_=at[:])
            ebias_t = []
            for i in range(4):
                bt = const_pool.tile([128, 1], f32, tag=f"ebias{i}",
                                     name=f"ebias{i}")
                nc.gpsimd.memset(bt[:], ebias[i])
                ebias_t.append(bt)

            weff_s = []
            weff = [None] * KT
            for j, (k0s, sw) in enumerate(SUPERS):
                w = weff_pool.tile([128, sw * OSH], f16, tag=f"weff{j}",
                                   name=f"weff{j}")
                weff_s.append(w)
                for s in range(sw):
                    weff[k0s + s] = w[:, ts(s, OSH)]

            # q/scl DMAs up front (deep-buffered; WAR edges pace reuse)
            qtls, sctls = [], []
            for j, (k0s, sw) in enumerate(SUPERS):
                qtl = deq_pool.tile([128, sw * OSH], i8, tag=f"qtile{sw}",
                                    name="qtl")
                sctl = deq_pool.tile([128, sw * OSH], f16, tag=f"sctile{sw}",
                                     name="sctl")
                for s in range(sw):
                    k = k0s + s
                    nc.gpsimd.dma_start(out=qtl[:, ts(s, OSH)],
                                        in_=qt[ts(k, 128), :])
                    nc.gpsimd.dma_start(out=sctl[:, ts(s, OSH)],
                                        in_=scl[ts(k, 128), :])
                qtls.append(qtl)
                sctls.append(sctl)

            def emit_supertile(j):
                """Emit the full dequant chain for supertile j:
                4x Exp (ACT), linear (DVE TS), LoRA wadd (PE) + x2 copies
                (ACT), sum + scale (DVE TT), weff fold (gpsimd)."""
                k0s, sw = SUPERS[j]
                Wj = sw * OSH
                es = []
                for i in range(4):
                    e = deq_pool.tile([128, Wj], f16, tag=f"e{i}_{sw}",
                                      name=f"e{i}")
                    nc.scalar.activation(e[:], qtls[j][:], Act.Exp,
                                         bias=ebias_t[i][:],
                                         scale=escale[i])
                    es.append(e)
                # LoRA: wadd2 = 2*(A@B) k-slab, via PE psum + ACT copy
                wadd2 = wadd2_pool.tile([128, Wj], f16, tag=f"wadd2_{sw}",
                                        name="wadd2")
                for s in range(sw):
                    k = k0s + s
                    p = wadd_pool.tile([128, OSH], f32, tag="waddps",
                                       name="waddps")
                    nc.tensor.matmul(p[:], at_sb[:, ts(k, 128)], b_sb[:],
                                     start=True, stop=True)
                    nc.scalar.activation(wadd2[:, ts(s, OSH)], p[:],
                                         Act.Copy, bias=0.0, scale=SCALING)
                # linear term on DVE (int8 in, fp16 out)
                lin = deq_pool.tile([128, Wj], f16, tag=f"lin{sw}",
                                    name="lin")
                nc.vector.tensor_scalar(lin[:], qtls[j][:], lscale, lbias,
                                        Alu.mult, Alu.add)
                # combine: acc = (pos exps) - (neg exps) + lin, on DVE
                pos = [es[i] for i in range(4) if esign[i] > 0]
                neg = [es[i] for i in range(4) if esign[i] < 0]
                nc.vector.tensor_tensor(pos[0][:], pos[0][:], pos[1][:],
                                        Alu.add)
                nc.vector.tensor_tensor(neg[0][:], neg[0][:], neg[1][:],
                                        Alu.add)
                nc.vector.tensor_tensor(pos[0][:], pos[0][:], neg[0][:],
                                        Alu.subtract)
                nc.vector.tensor_tensor(pos[0][:], pos[0][:], lin[:],
                                        Alu.add)
                # acc2 = acc * absmax_expanded
                acc2 = deq_pool.tile([128, Wj], f16, tag=f"acc2_{sw}",
                                     name="acc2")
                nc.vector.tensor_tensor(acc2[:], pos[0][:], sctls[j][:],
                                        Alu.mult)
                # weff = acc2 + wadd2  (gpsimd; frees DVE for next chain)
                nc.gpsimd.tensor_add(weff_s[j][:], acc2[:], wadd2[:])

            pending = list(range(NSW))

            def ensure_weff(k_need):
                while pending and SUPERS[pending[0]][0] <= k_need + AHEAD:
                    emit_supertile(pending.pop(0))

            # ---- Phase B: out[g*512+m*128 : ..., :] = x @ W_eff
            partials = {}

            def emit_chunk(g, ci, k0, k1, last):
                """Each chunk accumulates its own k-range in PSUM
                (start=True at k0); partial results combine on the DVE in
                f32 — no PE reinjection matmuls."""
                psums = [mm_pool.tile([128, OSH], f32, tag="mmps",
                                      name="mmps") for _ in range(MPG)]
                for k in range(k0, k1):
                    ensure_weff(k)
                    xg = x_pool.tile([128, TG], f16, tag="xg", name="xg")
                    nc.sync.dma_start(out=xg[:], in_=xt[ts(k, 128), ts(g, TG)])
                    for m in range(MPG):
                        nc.tensor.matmul(
                            psums[m][:],
                            xg[:, ts(m, 128)],
                            weff[k][:],
                            start=(k == k0),
                            stop=(k == k1 - 1))
                for m in range(MPG):
                    if ci == 0 and not last:
                        # first spill: plain copy on ACT (keeps DVE free
                        # for the dequant chains)
                        pt = part_pool.tile([128, OSH], f32,
                                            tag=f"part{g}_{m}",
                                            name=f"part{g}_{m}")
                        partials[(g, m)] = pt
                        nc.scalar.copy(pt[:], psums[m][:])
                    elif not last:
                        # accumulate into the partial (DVE, f32)
                        nc.vector.tensor_tensor(
                            partials[(g, m)][:], partials[(g, m)][:],
                            psums[m][:], Alu.add)
                    else:
                        ot = o_pool.tile([128, OSH], f16, tag="ot", name="ot")
                        if ci > 0:
                            nc.vector.tensor_tensor(
                                ot[:], psums[m][:], partials[(g, m)][:],
                                Alu.add)
                        elif m % 2 == 0:
                            nc.scalar.copy(ot[:], psums[m][:])
                        else:
                            nc.vector.tensor_scalar(ot[:], psums[m][:],
                                                    1.0, 0.0,
                                                    Alu.mult, Alu.add)
                        nc.scalar.dma_start(
                            out=out[ds(g * TG + m * 128, 128), :], in_=ot[:])

            # Wave order: by last tile needed (k1), then by FIRST tile
            # (k0) so big mostly-ready "absorber" chunks front-run the
            # short chunks that would otherwise stall on the wave's last
            # weff tiles.
            chunks = []      # (k1, k0, g, ci, last)
            for g in range(NG):
                b = CHUNKS.get(g, [0, KT])
                for ci in range(len(b) - 1):
                    chunks.append((b[ci + 1], b[ci], g, ci,
                                   ci == len(b) - 2))
            for k1, k0, g, ci, last in sorted(chunks):
                emit_chunk(g, ci, k0, k1, last)
            assert not pending

    nc.compile()
    _CACHE[key] = nc
    return nc


def _prepare_in_maps(x, q_idx, absmax, lora_A, lora_B):
    x = np.asarray(x, dtype=np.float32)
    q_idx = np.asarray(q_idx, dtype=np.int32)
    absmax = np.asarray(absmax, dtype=np.float32)
    lora_A = np.asarray(lora_A, dtype=np.float32)
    lora_B = np.asarray(lora_B, dtype=np.float32)

    xt = np.ascontiguousarray(x.reshape(TOK, IN).T.astype(np.float16))
    qt_full = q_idx.astype(np.int8).T                        # [IN, OUT] view
    at = np.ascontiguousarray(lora_A.T.astype(np.float16))   # [R, IN]

    in_maps = []
    for cid in range(NCORES):
        sl = slice(cid * OSH, (cid + 1) * OSH)
        scale = np.repeat(np.ascontiguousarray(absmax[sl].T), QBLOCK, axis=0)
        in_maps.append({
            "xt": xt,
            "qt": np.ascontiguousarray(qt_full[:, sl]),
            "scl": np.ascontiguousarray(scale.astype(np.float16)),
            "at": at,
            "bsh": np.ascontiguousarray(lora_B[:, sl].astype(np.float16)),
        })
    return in_maps


def _gather(results):
    shards = [results[cid]["out"] for cid in range(NCORES)]
    full = np.concatenate(shards, axis=1)                    # [TOK, OUT]
    return full.astype(np.float32).reshape(B_, S_, OUT)


def kernel(x, q_idx, absmax, lora_A, lora_B):
    from concourse.bass_utils import run_bass_kernel_spmd

    nc = _build()
    in_maps = _prepare_in_maps(x, q_idx, absmax, lora_A, lora_B)
    res = run_bass_kernel_spmd(nc, in_maps, list(range(NCORES)))
    return _gather(res.results)
